# revision 16
# baseline (speedup 1.0000x reference)
"""NodeAlignmentHead Trainium2 kernel.

Data-parallel over batch: 8 batches -> 8 NeuronCores, one [N,N] attention
block per core, no cross-core communication.

Math (per batch):
  v1n = z1 / ||z1||, v2n = z2 / ||z2||          (row-wise)
  S = v1n @ v2n.T  in [-1, 1]                   (cosine sims -> exp safe)
  att = exp(S) / rowsum(exp(S))
  out1 = z1 @ Wt + (att @ z2) @ Wb + b
  out2 = z2 @ Wt + (att.T @ z1) @ Wb + b
Folds used on device:
  out1 = isum * (expS @ (norm2*(v2n@Wb) + 1 x b)) + norm1*(v1n@Wt)
  out2.T = Wt.T @ z2.T + (isum*norm1*(v1n@Wb)).T @ expS + b
"""

import os
import sys
from contextlib import ExitStack

import numpy as np

sys.path.insert(0, "/opt/trn_rl_repo")

import concourse.bass as bass
import concourse.tile as tile
from concourse import bacc, masks, mybir
from concourse.bass_utils import run_bass_kernel_spmd

B, N, D = 8, 4096, 256
P = 128
NB = N // P  # 32 row blocks
FP32 = mybir.dt.float32
F32R = mybir.dt.float32r
BF16 = mybir.dt.bfloat16
AF = mybir.ActivationFunctionType
ALU = mybir.AluOpType

# stats tile column layout (one [128, 256] f32 tile)
SS1, SS2, NRM1, NRM2, INV1, INV2, ISUM, C1 = (slice(i * 32, (i + 1) * 32) for i in range(8))


def _col(sl, i):
    return slice(sl.start + i, sl.start + i + 1)


def build_module():
    nc = bacc.Bacc("TRN2", target_bir_lowering=False, debug=False)

    z1 = nc.dram_tensor("z1", [N, D], FP32, kind="ExternalInput").ap()
    z2 = nc.dram_tensor("z2", [N, D], FP32, kind="ExternalInput").ap()
    z2t = nc.dram_tensor("z2t", [D, N], FP32, kind="ExternalInput").ap()
    w = nc.dram_tensor("w", [2 * D, D], FP32, kind="ExternalInput").ap()
    bbc = nc.dram_tensor("bbc", [P, D], FP32, kind="ExternalInput").ap()
    bt = nc.dram_tensor("bt", [P, 2], FP32, kind="ExternalInput").ap()
    out1 = nc.dram_tensor("out1", [N, D], FP32, kind="ExternalOutput").ap()
    out2t = nc.dram_tensor("out2t", [D, N], FP32, kind="ExternalOutput").ap()
    exps_d = nc.dram_tensor("exps_scratch", [N, N], BF16).ap()

    with tile.TileContext(nc) as tc:
        _build_kernel(tc, z1, z2, z2t, w, bbc, bt, out1, out2t, exps_d)
    nc.compile()
    return nc


def _build_kernel(tc, z1, z2, z2t, w, bbc, bt, out1, out2t, exps_d):
    nc = tc.nc

    with ExitStack() as octx:
        # ---- long-lived pools ----
        const = octx.enter_context(tc.tile_pool(name="const", bufs=1))
        persist = octx.enter_context(tc.tile_pool(name="persist", bufs=1))

        ident = const.tile([P, P], FP32)
        masks.make_identity(nc, ident[:])
        wsb_raw = const.tile([P, 4 * D], FP32)  # c0,c1 = Wtop k-chunks; c2,c3 = Wbot
        nc.sync.dma_start(
            wsb_raw[:].rearrange("p (c d) -> p c d", c=4), w.rearrange("(c p) d -> p c d", p=P)
        )
        wsb = const.tile([P, 4 * D], F32R)
        nc.vector.tensor_copy(wsb[:], wsb_raw[:])
        bbc_sb = const.tile([P, D], FP32)
        nc.sync.dma_start(bbc_sb[:], bbc)
        bt_sb = const.tile([P, 2], FP32)
        nc.sync.dma_start(bt_sb[:], bt)

        stats = persist.tile([P, 256], FP32)
        z2bp = persist.tile([P, NB * D], BF16)   # z2b' by m-block
        z1cb = persist.tile([P, NB * D], BF16)   # isum*norm1*(v1n@Wb) by n-block
        w1 = persist.tile([P, NB * D], FP32)     # z1@Wt stash by n-block

        with ExitStack() as vctx:
            vtp = vctx.enter_context(tc.tile_pool(name="vtp", bufs=1))
            v1nT = vtp.tile([P, 2 * N], F32R)   # [d-chunk kc -> cols kc*N + n]
            v2nT = vtp.tile([P, 2 * N], F32R)

            # ================= pre-pass =================
            with ExitStack() as pctx:
                znp = pctx.enter_context(tc.tile_pool(name="znat", bufs=1))
                scr = pctx.enter_context(tc.tile_pool(name="prescr", bufs=2))
                pst = pctx.enter_context(tc.tile_pool(name="pstr", bufs=4, space="PSUM"))
                psz = pctx.enter_context(tc.tile_pool(name="psz", bufs=2, space="PSUM"))

                for zi, (zdram, ssc, nrmc, invc, vT) in enumerate(
                    [(z1, SS1, NRM1, INV1, v1nT), (z2, SS2, NRM2, INV2, v2nT)]
                ):
                    znat = znp.tile([P, NB * D], FP32, tag="znat")
                    nc.sync.dma_start(
                        znat[:].rearrange("p (t d) -> p t d", t=NB),
                        zdram.rearrange("(t p) d -> p t d", p=P),
                    )
                    for t in range(NB):
                        sq = scr.tile([P, D], FP32, tag="sq")
                        if zi == 0:
                            nc.vector.tensor_tensor_reduce(
                                sq[:], znat[:, t * D:(t + 1) * D], znat[:, t * D:(t + 1) * D],
                                1.0, 0.0, ALU.mult, ALU.add,
                                accum_out=stats[:, _col(ssc, t)],
                            )
                        else:
                            nc.scalar.activation(
                                sq[:], znat[:, t * D:(t + 1) * D], AF.Square,
                                accum_out=stats[:, _col(ssc, t)],
                            )
                    nc.scalar.activation(stats[:, nrmc], stats[:, ssc], AF.Sqrt)
                    nc.vector.tensor_scalar_max(stats[:, nrmc], stats[:, nrmc], 1e-12)
                    nc.vector.reciprocal(stats[:, invc], stats[:, nrmc])
                    for t in range(NB):
                        vn = scr.tile([P, D], FP32, tag="vn")
                        nc.vector.tensor_scalar_mul(
                            vn[:], znat[:, t * D:(t + 1) * D], stats[:, _col(invc, t)]
                        )
                        for kc in range(2):
                            ps = pst.tile([P, P], FP32, tag="trps")
                            nc.tensor.transpose(ps[:], vn[:, kc * P:(kc + 1) * P], ident[:])
                            nc.vector.tensor_copy(
                                vT[:, kc * N + t * P: kc * N + (t + 1) * P], ps[:]
                            )
                    if zi == 1:
                        # z2b' = norm2 * (v2n @ Wb) + b_broadcast   (bf16, by m-block)
                        for mb in range(NB):
                            pz = psz.tile([P, D], FP32, tag="pz")
                            for kc in range(2):
                                nc.tensor.matmul(
                                    pz[:],
                                    v2nT[:, kc * N + mb * P: kc * N + (mb + 1) * P],
                                    wsb[:, (2 + kc) * D:(3 + kc) * D],
                                    start=(kc == 0), stop=(kc == 1),
                                )
                            t2 = scr.tile([P, D], FP32, tag="t2")
                            nc.vector.tensor_scalar_mul(t2[:], pz[:], stats[:, _col(NRM2, mb)])
                            nc.vector.tensor_add(z2bp[:, mb * D:(mb + 1) * D], t2[:], bbc_sb[:])

            # ================= pass A: S, exp, row sums, z1cb, w1 =================
            with ExitStack() as actx:
                expp = actx.enter_context(tc.tile_pool(name="expp", bufs=2))
                rsp = actx.enter_context(tc.tile_pool(name="rsp", bufs=2))
                ps_s = actx.enter_context(tc.tile_pool(name="ps_s", bufs=3, space="PSUM"))
                ps_sm = actx.enter_context(tc.tile_pool(name="ps_sm", bufs=2, space="PSUM"))
                ascr = actx.enter_context(tc.tile_pool(name="ascr", bufs=2))

                for i in range(NB):
                    expS = expp.tile([P, N], BF16, tag="expS")
                    rs = rsp.tile([P, 4], FP32, tag="rs")
                    for q in range(4):
                        psq = ps_s.tile([P, 1024], FP32, tag="psq")
                        for c in range(2):
                            for kc in range(2):
                                nc.tensor.matmul(
                                    psq[:, c * 512:(c + 1) * 512],
                                    v1nT[:, kc * N + i * P: kc * N + (i + 1) * P],
                                    v2nT[:, kc * N + q * 1024 + c * 512: kc * N + q * 1024 + (c + 1) * 512],
                                    start=(kc == 0), stop=(kc == 1),
                                )
                        nc.scalar.activation(
                            expS[:, q * 1024:(q + 1) * 1024], psq[:], AF.Exp,
                            accum_out=rs[:, q:q + 1],
                        )
                    nc.gpsimd.dma_start(exps_d[i * P:(i + 1) * P, :], expS[:])
                    rsum = ascr.tile([P, 1], FP32, tag="rsum")
                    nc.vector.reduce_sum(rsum[:], rs[:], axis=mybir.AxisListType.X)
                    nc.vector.reciprocal(stats[:, _col(ISUM, i)], rsum[:])
                    nc.vector.tensor_mul(
                        stats[:, _col(C1, i)], stats[:, _col(ISUM, i)], stats[:, _col(NRM1, i)]
                    )
                    # z1cb_i (bf16) and w1_i (f32)
                    pz1 = ps_sm.tile([P, D], FP32, tag="pz1")
                    pw1 = ps_sm.tile([P, D], FP32, tag="pz1")
                    for kc in range(2):
                        nc.tensor.matmul(
                            pz1[:],
                            v1nT[:, kc * N + i * P: kc * N + (i + 1) * P],
                            wsb[:, (2 + kc) * D:(3 + kc) * D],
                            start=(kc == 0), stop=(kc == 1),
                        )
                        nc.tensor.matmul(
                            pw1[:],
                            v1nT[:, kc * N + i * P: kc * N + (i + 1) * P],
                            wsb[:, kc * D:(kc + 1) * D],
                            start=(kc == 0), stop=(kc == 1),
                        )
                    nc.vector.tensor_scalar_mul(
                        z1cb[:, i * D:(i + 1) * D], pz1[:], stats[:, _col(C1, i)]
                    )
                    nc.vector.tensor_scalar_mul(
                        w1[:, i * D:(i + 1) * D], pw1[:], stats[:, _col(NRM1, i)]
                    )

        # ================= pass B1: out2.T =================
        with ExitStack() as bctx:
            z2tp = bctx.enter_context(tc.tile_pool(name="z2tp", bufs=1))
            enp = bctx.enter_context(tc.tile_pool(name="enp", bufs=4))
            o2p = bctx.enter_context(tc.tile_pool(name="o2p", bufs=2))
            ps_o2 = bctx.enter_context(tc.tile_pool(name="ps_o2", bufs=2, space="PSUM"))

            z2tsb_raw = z2tp.tile([P, 2 * N], FP32)
            nc.sync.dma_start(
                z2tsb_raw[:].rearrange("p (c n) -> p c n", c=2),
                z2t.rearrange("(c p) n -> p c n", p=P),
            )
            z2tsb = z2tp.tile([P, 2 * N], F32R)
            nc.vector.tensor_copy(z2tsb[:], z2tsb_raw[:])

            for h in range(2):
                ps = [ps_o2.tile([P, 2048], FP32, tag="pso2", name=f"pso2_{h}_{j}") for j in range(2)]
                for dc in range(2):
                    for kc in range(2):
                        for s in range(4):
                            nc.tensor.matmul(
                                ps[dc][:, s * 512:(s + 1) * 512],
                                wsb[:, kc * D + dc * P: kc * D + dc * P + P],
                                z2tsb[:, kc * N + h * 2048 + s * 512: kc * N + h * 2048 + (s + 1) * 512],
                                start=(kc == 0), stop=False,
                            )
                for nb in range(NB):
                    en = enp.tile([P, 2048], BF16, tag="en")
                    (nc.gpsimd if nb % 2 == 0 else nc.sync).dma_start(en[:], exps_d[nb * P:(nb + 1) * P, h * 2048:(h + 1) * 2048])
                    for dc in range(2):
                        for s in range(4):
                            nc.tensor.matmul(
                                ps[dc][:, s * 512:(s + 1) * 512],
                                z1cb[:, nb * D + dc * P: nb * D + dc * P + P],
                                en[:, s * 512:(s + 1) * 512],
                                start=False, stop=(nb == NB - 1),
                            )
                for dc in range(2):
                    o2t = o2p.tile([P, 2048], FP32, tag="o2t")
                    nc.scalar.activation(o2t[:], ps[dc][:], AF.Identity, bias=bt_sb[:, dc:dc + 1])
                    nc.gpsimd.dma_start(out2t[dc * P:(dc + 1) * P, h * 2048:(h + 1) * 2048], o2t[:])

        # ================= pass B2: out1 (single pass over expS.T) =================
        # Stream expS.T once; accumulate 8-mc partial sums in scratch PSUM, then
        # fold into an SBUF accumulator (DVE work / 8).
        CH = 8
        with ExitStack() as cctx:
            etp = cctx.enter_context(tc.tile_pool(name="etp", bufs=CH + 2))
            o1p = cctx.enter_context(tc.tile_pool(name="o1p", bufs=3))
            tmpp = cctx.enter_context(tc.tile_pool(name="tmpp", bufs=3))
            accp = cctx.enter_context(tc.tile_pool(name="accp", bufs=1))
            ps_sc = cctx.enter_context(tc.tile_pool(name="ps_sc", bufs=4, space="PSUM"))

            acc = accp.tile([P, NB * D], FP32)
            for ch in range(NB // CH):
                ets = []
                for k in range(CH):
                    mc = ch * CH + k
                    et = etp.tile([P, N], BF16, tag="et", name=f"et_{mc}")
                    eng = nc.sync if (mc % 2 == 0) else nc.scalar
                    eng.dma_start(et[:], exps_d[:, mc * P:(mc + 1) * P], transpose=True)
                    ets.append(et)
                for j in range(NB // 2):
                    sc = ps_sc.tile([P, 512], FP32, tag="psc", name=f"psc_{ch}_{j}")
                    for k in range(CH):
                        mc = ch * CH + k
                        for half in range(2):
                            i = j * 2 + half
                            nc.tensor.matmul(
                                sc[:, half * D:(half + 1) * D],
                                ets[k][:, i * P:(i + 1) * P],
                                z2bp[:, mc * D:(mc + 1) * D],
                                start=(k == 0 and half == 0),
                                stop=(k == CH - 1 and half == 1),
                            )
                    aslc = acc[:, j * 512:(j + 1) * 512]
                    if ch == 0:
                        nc.vector.tensor_copy(aslc, sc[:])
                    else:
                        nc.vector.tensor_add(aslc, aslc, sc[:])
            for i in range(NB):
                tmp = tmpp.tile([P, D], FP32, tag="tmp")
                nc.scalar.activation(
                    tmp[:], acc[:, i * D:(i + 1) * D], AF.Copy, scale=stats[:, _col(ISUM, i)]
                )
                o1t = o1p.tile([P, D], FP32, tag="o1t")
                nc.vector.tensor_add(o1t[:], tmp[:], w1[:, i * D:(i + 1) * D])
                nc.gpsimd.dma_start(out1[i * P:(i + 1) * P, :], o1t[:])
